# revision 24
# baseline (speedup 1.0000x reference)
"""Trainium2 Bass kernel for nn_ClearMeshLoss (8-core SPMD), v4.

Strategy (v4):
  - chamfer + normal-consistency: both clouds sorted by x on host. Each core
    owns 1250 consecutive sorted query rows (10 strips of 128) per side.
    Each strip scores a rank-aligned window of W=512 sorted target columns
    (+-1e9 x sentinels) with the augmented matmul c = 2*a.b - |b|^2 in
    bf16 hi/lo (K=11).  Matmuls are 4-way row-tiled (tile_position row
    groups) so up to 4 strips stream concurrently through the PE.  Strips
    are paired (A_k with B_{k+1}) into 2-bank PSUM tiles; evacuation is
    split: V-pairs are max-reduced to 32-wide subtile maxes on DVE (f32),
    S-pairs are copied PSUM->SBUF as bf16 by ScalarE and DMA'd raw to HBM
    where the host computes the subtile maxes.  Host picks top-2 subtiles
    per row, recomputes those 64 candidate distances exactly, PROVES
    optimality via the x-gap bound + an epsilon-aware bound over all
    subtiles, and falls back to an exact KD-tree query for rows failing
    the proof.  Exact for this input.
  - edge loss: cross products / dots in bf16 on DVE with a component-
    duplicated [x,y,z,x,y] layout (rotations become plain slices);
    sqrt+relu on ScalarE (sqrt table preloaded early); host does the
    integer edge pairing.
  - sdf: clips/sub on DVE (bf16), abs/exp + accumulations on ScalarE.
  - eikonal: finite diffs on DVE from the same sdf tiles (no extra DMA);
    row-border pairs are patched exactly on host.
"""
import numpy as np
import ml_dtypes

BF16 = np.dtype(ml_dtypes.bfloat16)

# ---------------------------------------------------------------- constants
SDF_W, EIK_W, CH_W, NORM_W, EDGE_W, WT_W = 1.0, 0.1, 1.0, 0.5, 0.3, 0.2
TRUNC, SURF_W, DIH_THR = 0.1, 5.0, 0.5
SIGMA = TRUNC / 3.0

N_CORES = 8

FULL_CFG = dict(
    npts=10000,
    shard=1250,
    n_strips=10,          # strips of 128 rows per side
    win=512,
    sub=32,
    padl=192,             # left sentinels in ext array
    ext_len=192 + 10000 + 222,
    slice_w=9 * 128 + 512,    # 1664
    v_pairs=(2, 7),       # pair ids reduced on-device (DVE); rest raw bf16
    sdf_f=196,
    sdf_shard=25000,
    pair_f=120,
    pair_cap=8 * 128 * 120,
)
FULL_CFG["nsub"] = FULL_CFG["win"] // FULL_CFG["sub"]

_PROG_CACHE = {}


def _pair_strips(cfg):
    """pair p -> [(side,k) j=0, (side,k) j=1]; A_k paired with B_{k+1} so
    adjacent matmuls hit different PE row groups."""
    n = cfg["n_strips"]
    return [[(0, p), (1, (p + 1) % n)] for p in range(n)]


def build_program(cfg):
    from contextlib import ExitStack
    import concourse.bacc as bacc
    import concourse.bass as bass
    import concourse.tile as tile
    from concourse import mybir

    f32 = mybir.dt.float32
    bf16 = mybir.dt.bfloat16
    AX = mybir.AxisListType
    OP = mybir.AluOpType
    AF = mybir.ActivationFunctionType

    n_strips = cfg["n_strips"]
    win = cfg["win"]
    sub = cfg["sub"]
    nsub = cfg["nsub"]
    slice_w = cfg["slice_w"]
    sdf_f = cfg["sdf_f"]
    P = cfg["pair_f"]
    pairs = _pair_strips(cfg)
    v_pairs = set(cfg["v_pairs"])
    NVp = len(v_pairs)
    NSp = n_strips - NVp

    # group-g strip lists (strips s with s%4==g), per side
    g_strips = [[s for s in range(n_strips) if s % 4 == g] for g in range(4)]

    nc = bacc.Bacc("TRN2", target_bir_lowering=False)

    # ---- dram inputs ----
    d_lhs = [nc.dram_tensor(f"lhs_g{g}", [11, 2 * len(g_strips[g]) * 128],
                            bf16, kind="ExternalInput") for g in range(4)]
    d_rhs = nc.dram_tensor("rhs_ab", [11, 2 * slice_w], bf16, kind="ExternalInput")
    d_sdf = nc.dram_tensor("sdf_pg", [128, 2 * sdf_f], bf16, kind="ExternalInput")
    d_edge = nc.dram_tensor("edge_in", [128, 24 * P], bf16, kind="ExternalInput")

    # ---- dram outputs ----
    d_chams = nc.dram_tensor("cham_s", [128, NSp * 2 * win], bf16,
                             kind="ExternalOutput")
    # cham_v then part cols: 0 sdf_absdiff, 1 sdf_dead, 2 eik_num, 3 eik_cnt,
    # 4 edge_relu
    d_out = nc.dram_tensor("out_vp", [128, NVp * 2 * nsub + 8], f32,
                           kind="ExternalOutput")

    with tile.TileContext(nc) as tc, ExitStack() as ctx:
        sing = ctx.enter_context(tc.tile_pool(name="sing", bufs=1))
        epool = ctx.enter_context(tc.tile_pool(name="epool", bufs=1))
        spool = ctx.enter_context(tc.tile_pool(name="spool", bufs=1))
        psum = ctx.enter_context(tc.tile_pool(name="psum", bufs=4, space="PSUM"))

        NVs = NVp * 2 * nsub
        out_vp = sing.tile([128, NVs + 8], f32)

        def part_col(c):
            return out_vp[:, NVs + c:NVs + c + 1]

        chams_o = sing.tile([128, NSp, 2, win], bf16)

        nc.vector.memset(out_vp[:, NVs:NVs + 8], 0.0)
        sbias = epool.tile([128, 1], f32)
        nbias = epool.tile([128, 1], f32)
        sqdummy = epool.tile([128, 1], f32)
        nc.vector.memset(sbias, 1e-30)
        nc.vector.memset(nbias, -DIH_THR)

        # ---- input DMAs (per-group tiles so MMs start on first arrivals) ----
        lhs_sb = [sing.tile([128, 2 * len(g_strips[g]) * 128], bf16,
                            name=f"lhs_sb{g}") for g in range(4)]
        rhs_sb = [sing.tile([128, 2 * slice_w], bf16, name=f"rhs_sb{g}")
                  for g in range(4)]
        for g in range(4):
            ng = len(g_strips[g])
            nc.sync.dma_start(out=rhs_sb[g][32 * g:32 * g + 11, :],
                              in_=d_rhs[:, :])
            nc.gpsimd.dma_start(
                out=lhs_sb[g][32 * g:32 * g + 11, 0:2 * ng * 128],
                in_=d_lhs[g][:, :])

        sdf_sb = spool.tile([128, 2 * sdf_f], bf16)
        sp = sdf_sb[:, 0:sdf_f]
        sg = sdf_sb[:, sdf_f:2 * sdf_f]
        nc.sync.dma_start(out=sdf_sb, in_=d_sdf[:, :])

        pl = epool.tile([128, 24, P], bf16)
        nc.scalar.dma_start(out=pl, in_=d_edge[:, :])
        # preload the sqrt ACT table while ScalarE would otherwise idle
        nc.scalar.activation(out=sqdummy, in_=sbias[:, 0:1], func=AF.Sqrt,
                             bias=sbias[:, 0:1])

        # ---- matmuls: 10 pairs x 2 strips, 4-way row-tiled ----
        ps_tiles = []
        for p in range(n_strips):
            ps = psum.tile([128, 2, win], f32)
            for j, (side, k) in enumerate(pairs[p]):
                g, t = k % 4, k // 4
                lcol = (side * len(g_strips[g]) + t) * 128
                rcol = side * slice_w + k * 128
                nc.tensor.matmul(ps[:, j, :],
                                 lhs_sb[g][32 * g:32 * g + 11, lcol:lcol + 128],
                                 rhs_sb[g][32 * g:32 * g + 11, rcol:rcol + win],
                                 start=True, stop=True,
                                 tile_position=(32 * g, 0))
            ps_tiles.append(ps)

        # ---- sdf elementwise (DVE bf16) ----
        prc = spool.tile([128, sdf_f], bf16)
        gc = spool.tile([128, sdf_f], bf16)
        nc.vector.tensor_scalar(out=prc, in0=sp, scalar1=TRUNC, scalar2=-TRUNC,
                                op0=OP.min, op1=OP.max)
        nc.vector.tensor_scalar(out=gc, in0=sg, scalar1=TRUNC, scalar2=-TRUNC,
                                op0=OP.min, op1=OP.max)
        diff = spool.tile([128, sdf_f], bf16)
        nc.vector.tensor_tensor(out=diff, in0=prc, in1=gc, op=OP.subtract)

        # Scalar: abs-diff accum, weights
        absdiff = spool.tile([128, sdf_f], bf16)
        nc.scalar.activation(out=absdiff, in_=diff, func=AF.Abs,
                             accum_out=part_col(0))
        absg = spool.tile([128, sdf_f], bf16)
        nc.scalar.activation(out=absg, in_=gc, func=AF.Abs)
        e4 = spool.tile([128, sdf_f], bf16)
        nc.scalar.activation(out=e4, in_=absg, func=AF.Exp, scale=-1.0 / SIGMA)

        # ---- eikonal (DVE; GpSimd only poisons the mask column) ----
        F1 = sdf_f - 1
        dx = spool.tile([128, F1], bf16)
        nc.vector.tensor_tensor(out=dx, in0=sp[:, 1:sdf_f], in1=sp[:, 0:F1],
                                op=OP.subtract)
        ndx = spool.tile([128, F1], bf16)
        nc.vector.tensor_scalar(out=ndx, in0=dx, scalar1=-1.0, scalar2=None,
                                op0=OP.mult)
        absdx = spool.tile([128, F1], bf16)
        nc.vector.tensor_tensor(out=absdx, in0=dx, in1=ndx, op=OP.max)
        # poison column j=107 so the shard-boundary pair (row 127) is masked
        # out; host exactly re-adds the 127 real pairs this also kills
        nc.gpsimd.memset(absg[:, 107:108], 1.0)

        # ======== evacuation + edge, interleaved for engine overlap ========
        v_slot, s_slot = {}, {}
        for p in range(n_strips):
            if p in v_pairs:
                v_slot[p] = len(v_slot)
            else:
                s_slot[p] = len(s_slot)

        def evac(p):
            ps = ps_tiles[p]
            if p in v_pairs:
                vi = v_slot[p]
                ps_ap = ps[:, :, :]
                ps4d = bass.AP(tensor=ps_ap.tensor, offset=ps_ap.offset,
                               ap=[ps_ap.ap[0], [win, 2], [sub, nsub],
                                   [1, sub]])
                nc.vector.tensor_reduce(
                    out=out_vp[:, vi * 2 * nsub:(vi + 1) * 2 * nsub],
                    in_=ps4d, axis=AX.X, op=OP.max)
            else:
                si = s_slot[p]
                nc.scalar.activation(out=chams_o[:, si, :, :], in_=ps[:, :, :],
                                     func=AF.Copy)

        # edge tiles; E5 holds each edge vector with components [x,y,z,x,y]
        # so rot1/rot2 are plain slices (comps 1:4 / 2:5).
        E5 = epool.tile([128, 4, 5, P], bf16)     # e1A,e2A,e1B,e2B
        T1 = epool.tile([128, 2, 3, P], bf16)
        T2 = epool.tile([128, 2, 3, P], bf16)
        NN = epool.tile([128, 2, 3, P], bf16)
        SS = epool.tile([128, 3, 3, P], bf16)     # [na^2, nb^2, na*nb] comps
        A1 = epool.tile([128, 3, P], bf16)
        DOTS = epool.tile([128, 3, P], bf16)
        den2 = epool.tile([128, P], bf16)
        sa = epool.tile([128, P], f32)
        rs = epool.tile([128, P], f32)
        cosb = epool.tile([128, P], f32)
        relu_d = epool.tile([128, P], f32)

        plb = pl[:, :, :]
        e5b = E5[:, :, :, :]

        def pl_ap(plane0, ncomp):
            return bass.AP(tensor=plb.tensor, offset=plb.offset + plane0 * P,
                           ap=[plb.ap[0], [3 * P, 4], [P, ncomp], [1, P]])

        def e5_ap(comp0, ncomp, vstep=1, v0=0, nvec=4):
            return bass.AP(tensor=e5b.tensor,
                           offset=e5b.offset + (v0 * 5 + comp0) * P,
                           ap=[e5b.ap[0], [vstep * 5 * P, nvec], [P, ncomp],
                               [1, P]])

        edge_ops = []
        # plane order: v1A v2A v1B v2B (0:12) then v0A v0A v0B v0B (12:24)
        edge_ops.append(lambda: nc.vector.tensor_tensor(
            out=e5_ap(0, 3), in0=pl_ap(0, 3), in1=pl_ap(12, 3), op=OP.subtract))
        edge_ops.append(lambda: nc.vector.tensor_tensor(
            out=e5_ap(3, 2), in0=pl_ap(0, 2), in1=pl_ap(12, 2), op=OP.subtract))
        # crosses: na = e1A_r1*e2A_r2 - e1A_r2*e2A_r1 ; nb likewise
        edge_ops.append(lambda: nc.vector.tensor_tensor(
            out=T1[:, :, :, :], in0=e5_ap(1, 3, 2, 0, 2),
            in1=e5_ap(2, 3, 2, 1, 2), op=OP.mult))
        edge_ops.append(lambda: nc.vector.tensor_tensor(
            out=T2[:, :, :, :], in0=e5_ap(2, 3, 2, 0, 2),
            in1=e5_ap(1, 3, 2, 1, 2), op=OP.mult))
        edge_ops.append(lambda: nc.vector.tensor_tensor(
            out=NN[:, :, :, :], in0=T1[:, :, :, :], in1=T2[:, :, :, :],
            op=OP.subtract))
        # dots
        edge_ops.append(lambda: nc.vector.tensor_tensor(
            out=SS[:, 0:2, :, :], in0=NN[:, :, :, :], in1=NN[:, :, :, :],
            op=OP.mult))
        edge_ops.append(lambda: nc.vector.tensor_tensor(
            out=SS[:, 2, :, :], in0=NN[:, 0, :, :], in1=NN[:, 1, :, :],
            op=OP.mult))
        edge_ops.append(lambda: nc.vector.tensor_tensor(
            out=A1[:, :, :], in0=SS[:, :, 0, :], in1=SS[:, :, 1, :], op=OP.add))
        edge_ops.append(lambda: nc.vector.tensor_tensor(
            out=DOTS[:, :, :], in0=A1[:, :, :], in1=SS[:, :, 2, :], op=OP.add))
        edge_ops.append(lambda: nc.vector.tensor_tensor(
            out=den2, in0=DOTS[:, 0, :], in1=DOTS[:, 1, :], op=OP.mult))

        # interleave: pair evacuations with edge ops slotted in after pair 2
        edge_iter = iter(edge_ops)
        for p in range(n_strips):
            evac(p)
            if p >= 2:
                for _ in range(2):
                    op = next(edge_iter, None)
                    if op is not None:
                        op()
        for op in edge_iter:
            op()

        # cham_s chunk DMAs (ordered after their producing copies by deps)
        h1 = (NSp + 1) // 2
        nc.gpsimd.dma_start(out=d_chams[:, 0:h1 * 2 * win],
                            in_=chams_o[:, 0:h1, :, :])
        nc.gpsimd.dma_start(out=d_chams[:, h1 * 2 * win:NSp * 2 * win],
                            in_=chams_o[:, h1:NSp, :, :])

        # edge tail
        nc.scalar.activation(out=sa, in_=den2, func=AF.Sqrt, bias=sbias[:, 0:1])
        nc.vector.reciprocal_approx_fast(out=rs, in_=sa)
        nc.vector.tensor_tensor(out=cosb, in0=DOTS[:, 2, :], in1=rs, op=OP.mult)
        nc.scalar.activation(out=relu_d, in_=cosb, func=AF.Relu,
                             bias=nbias[:, 0:1], accum_out=part_col(4))

        # eik elementwise tail (DVE) + accumulators
        t_ = spool.tile([128, F1], bf16)
        nc.vector.tensor_scalar(out=t_, in0=absdx, scalar1=-1.0, scalar2=None,
                                op0=OP.add)
        mask = spool.tile([128, F1], bf16)
        nc.vector.tensor_scalar(out=mask, in0=absg[:, 0:F1], scalar1=TRUNC,
                                scalar2=None, op0=OP.is_lt)
        tm = spool.tile([128, F1], bf16)
        nc.vector.tensor_tensor(out=tm, in0=t_, in1=mask, op=OP.mult)
        nc.vector.tensor_reduce(out=part_col(3), in_=mask, axis=AX.X, op=OP.add)
        deadd = spool.tile([128, sdf_f], bf16)
        nc.vector.scalar_tensor_tensor(out=deadd, in0=e4, scalar=SURF_W - 1.0,
                                       in1=absdiff, op0=OP.mult, op1=OP.mult,
                                       accum_out=part_col(1))
        eikd = spool.tile([128, F1], bf16)
        nc.vector.scalar_tensor_tensor(out=eikd, in0=tm, scalar=1.0,
                                       in1=t_, op0=OP.mult, op1=OP.mult,
                                       accum_out=part_col(2))

        nc.sync.dma_start(out=d_out[:, :], in_=out_vp[:, :])

    nc.compile()
    return nc


def get_program(cfg_key="full"):
    if cfg_key not in _PROG_CACHE:
        _PROG_CACHE[cfg_key] = build_program(FULL_CFG)
    return _PROG_CACHE[cfg_key]


# ================================================================== host side
def _hi_lo(x):
    h = x.astype(BF16)
    l = (x - h.astype(np.float64)).astype(BF16)
    return h, l


def _build_lhs(a):
    """a: [n,3] fp64 -> [11,n] bf16 rows [ah3, ah3, al3, 1, 1]."""
    ah, al = _hi_lo(a)
    ones = np.ones((1, a.shape[0]), BF16)
    return np.ascontiguousarray(np.concatenate([ah.T, ah.T, al.T, ones, ones], 0))


def _build_rhs(b):
    """b: [m,3] fp64 -> [11,m] bf16 rows [2bh3, 2bl3, 2bh3, -sh, -sl]."""
    bh = b.astype(BF16)
    bl2 = (2.0 * (b - bh.astype(np.float64))).astype(BF16)
    bh2 = (2.0 * bh.astype(np.float64)).astype(BF16)
    s = (b * b).sum(-1)
    sh = s.astype(BF16)
    sl = (s - sh.astype(np.float64)).astype(BF16)
    neg_sh = (-sh.astype(np.float64)).astype(BF16)
    neg_sl = (-sl.astype(np.float64)).astype(BF16)
    return np.ascontiguousarray(
        np.concatenate([bh2.T, bl2.T, bh2.T, neg_sh[None], neg_sl[None]], 0))


def _pad_rows(x, n):
    out = np.zeros((n, 3))
    out[:x.shape[0]] = x
    return out


def _host_prep(inputs, cfg):
    np_f32 = np.float32
    npts = cfg["npts"]
    shard = cfg["shard"]
    n_strips = cfg["n_strips"]
    slice_w = cfg["slice_w"]
    padl = cfg["padl"]
    ext_len = cfg["ext_len"]
    sdf_f = cfg["sdf_f"]
    sdf_shard = cfg["sdf_shard"]
    P = cfg["pair_f"]

    pred_pts = np.asarray(inputs["pred_points"][0], dtype=np.float64)
    gt_pts = np.asarray(inputs["gt_points"][0], dtype=np.float64)

    pperm = np.argsort(pred_pts[:, 0], kind="stable")
    gperm = np.argsort(gt_pts[:, 0], kind="stable")
    ps = pred_pts[pperm]
    gs = gt_pts[gperm]

    def make_ext(sorted_pts):
        ext = np.empty((ext_len, 3))
        ext[:padl] = [-1e9, 0.0, 0.0]
        ext[padl:padl + npts] = sorted_pts
        ext[padl + npts:] = [1e9, 0.0, 0.0]
        return ext

    g_ext = make_ext(gs)
    p_ext = make_ext(ps)
    rhs_gt = _build_rhs(g_ext)     # [11, ext_len]
    rhs_pr = _build_rhs(p_ext)

    pred_sdf = inputs["pred_sdf"].reshape(-1).astype(np_f32)
    gt_sdf = inputs["gt_sdf"].reshape(-1).astype(np_f32)

    # --- edge pairing on host (int32 faces only) ---
    verts = np.asarray(inputs["extracted_vertices"], dtype=np_f32)
    faces = np.asarray(inputs["extracted_faces"], dtype=np.int64)
    V = verts.shape[0]
    Fn = faces.shape[0]
    a = faces
    b = np.roll(faces, -1, axis=1)
    lo = np.minimum(a, b)
    hi = np.maximum(a, b)
    key = (lo * V + hi).reshape(-1)
    fid = np.repeat(np.arange(Fn, dtype=np.int64), 3)
    order = np.argsort(key, kind="stable")
    k = key[order]
    f = fid[order]
    same_next = k[:-1] == k[1:]
    prev = np.concatenate([[False], same_next[:-1]])
    nxt = np.concatenate([same_next[1:], [False]])
    is_pair = same_next & ~prev & ~nxt
    pos = np.nonzero(is_pair)[0]
    fa = f[pos]
    fb = f[pos + 1]
    npairs = int(pos.shape[0])
    is_start = np.concatenate([[True], k[1:] != k[:-1]])
    starts = np.nonzero(is_start)[0]
    run_len = np.diff(np.concatenate([starts, [k.shape[0]]]))
    total_unique = int(starts.shape[0])
    bad = int((run_len != 2).sum())
    wt = (bad / total_unique) if total_unique > 0 else 0.0

    pair_cap = cfg["pair_cap"]
    n_dev = min(npairs, pair_cap)
    # plane order: v1A v2A v1B v2B | v0A v0A v0B v0B (each 3 comps)
    planes = np.zeros((24, pair_cap), np_f32)
    if n_dev > 0:
        va = verts[faces[fa[:n_dev]]]     # [n,3(vert),3(comp)]
        vb = verts[faces[fb[:n_dev]]]
        planes[0:3, :n_dev] = va[:, 1].T
        planes[3:6, :n_dev] = va[:, 2].T
        planes[6:9, :n_dev] = vb[:, 1].T
        planes[9:12, :n_dev] = vb[:, 2].T
        planes[12:15, :n_dev] = va[:, 0].T
        planes[15:18, :n_dev] = va[:, 0].T
        planes[18:21, :n_dev] = vb[:, 0].T
        planes[21:24, :n_dev] = vb[:, 0].T
    edge_extra = 0.0
    if npairs > pair_cap:
        va = verts[faces[fa[pair_cap:]]]
        vb = verts[faces[fb[pair_cap:]]]
        na = np.cross(va[:, 1] - va[:, 0], va[:, 2] - va[:, 0])
        nb = np.cross(vb[:, 1] - vb[:, 0], vb[:, 2] - vb[:, 0])
        na /= np.maximum(np.linalg.norm(na, axis=-1, keepdims=True), 1e-12)
        nb /= np.maximum(np.linalg.norm(nb, axis=-1, keepdims=True), 1e-12)
        cosv = (na * nb).sum(-1)
        edge_extra = float(np.maximum(cosv - DIH_THR, 0.0).sum())
    planes_bf = planes.astype(BF16)
    planes8 = planes_bf.reshape(24, N_CORES, 128, P).transpose(1, 2, 0, 3)

    g_strips = [[s for s in range(n_strips) if s % 4 == g] for g in range(4)]

    in_maps = []
    sdf_tiles_p, sdf_tiles_g = [], []
    for c in range(N_CORES):
        lhs_a = _build_lhs(_pad_rows(ps[c * shard:(c + 1) * shard], 128 * n_strips))
        lhs_b = _build_lhs(_pad_rows(gs[c * shard:(c + 1) * shard], 128 * n_strips))
        im = {}
        for g in range(4):
            ng = len(g_strips[g])
            blk = np.empty((11, 2 * ng * 128), BF16)
            for side, lhs in ((0, lhs_a), (1, lhs_b)):
                for t, s in enumerate(g_strips[g]):
                    blk[:, (side * ng + t) * 128:(side * ng + t + 1) * 128] = \
                        lhs[:, s * 128:(s + 1) * 128]
            im[f"lhs_g{g}"] = np.ascontiguousarray(blk)
        im["rhs_ab"] = np.ascontiguousarray(np.concatenate(
            [rhs_gt[:, c * shard:c * shard + slice_w],
             rhs_pr[:, c * shard:c * shard + slice_w]], axis=1))

        spd = np.full(128 * sdf_f, 1e9, np_f32)
        sgd = np.full(128 * sdf_f, 1e9, np_f32)
        sl = pred_sdf[c * sdf_shard:(c + 1) * sdf_shard]
        spd[:sl.shape[0]] = sl
        sgd[:sl.shape[0]] = gt_sdf[c * sdf_shard:(c + 1) * sdf_shard]
        spd_bf = spd.astype(BF16).reshape(128, sdf_f)
        sgd_bf = sgd.astype(BF16).reshape(128, sdf_f)
        im["sdf_pg"] = np.ascontiguousarray(
            np.concatenate([spd_bf, sgd_bf], axis=1))
        sdf_tiles_p.append(spd_bf)
        sdf_tiles_g.append(sgd_bf)

        im["edge_in"] = np.ascontiguousarray(planes8[c].reshape(128, 24 * P))
        in_maps.append(im)

    meta = dict(npairs=npairs, wt=wt, edge_extra=edge_extra,
                pperm=pperm, gperm=gperm, ps=ps, gs=gs,
                p_ext=p_ext, g_ext=g_ext,
                sdf_p=sdf_tiles_p, sdf_g=sdf_tiles_g,
                pred_sdf=pred_sdf, gt_sdf=gt_sdf)
    return in_maps, meta


def _eik_host_corrections(cfg, meta):
    """Row-border dx pairs the device skips + the poisoned mask column,
    computed with the same bf16-input/f32-arith convention."""
    sdf_f, sdf_shard = cfg["sdf_f"], cfg["sdf_shard"]
    n_batch = 100000
    n_tot = 200000
    num_add = 0.0
    cnt_add = 0.0
    for c in range(N_CORES):
        spd = meta["sdf_p"][c].reshape(-1).astype(np.float32)
        sgd = meta["sdf_g"][c].reshape(-1).astype(np.float32)
        # (a) row borders (L = 196p+195) + poisoned column (L = 196p+107),
        #     p in [0, 126]
        p = np.arange(127)
        L = np.concatenate([sdf_f * p + (sdf_f - 1), sdf_f * p + 107])
        ok = L + 1 <= sdf_shard - 1
        L = L[ok]
        i_glob = c * sdf_shard + L
        valid = (i_glob % n_batch) != n_batch - 1
        dxv = spd[L + 1] - spd[L]
        tv = np.abs(dxv) - 1.0
        mk = (np.abs(sgd[L]) < TRUNC) & valid
        num_add += float((tv * tv * mk).sum())
        cnt_add += float(mk.sum())
        # (b) poisoned slot L=24999: pair crosses into next core's shard
        L = sdf_shard - 1
        i_glob = c * sdf_shard + L
        if i_glob + 1 < n_tot and (i_glob % n_batch) != n_batch - 1:
            nxt2 = meta["pred_sdf"][(c + 1) * sdf_shard]
            nxt2 = np.float32(np.asarray(nxt2, np.float32).astype(BF16))
            dxv = nxt2 - spd[L]
            tv = np.abs(dxv) - 1.0
            mk = np.abs(sgd[L]) < TRUNC
            if mk:
                num_add += float(tv * tv)
                cnt_add += 1.0
    return num_add, cnt_add


def _exact_nn(q, t_sorted):
    try:
        from scipy.spatial import cKDTree
        tree = cKDTree(t_sorted)
        d, idx = tree.query(q, k=1)
        return d * d, idx
    except Exception:
        n = q.shape[0]
        dm = np.empty(n)
        im = np.empty(n, np.int64)
        B = 512
        for i in range(0, n, B):
            d2 = ((q[i:i + B, None, :] - t_sorted[None, :, :]) ** 2).sum(-1)
            im[i:i + B] = np.argmin(d2, 1)
            dm[i:i + B] = d2[np.arange(d2.shape[0]), im[i:i + B]]
        return dm, im


def _cham_side(cfg, rr, eps, qs, ext, t_sorted, a2):
    """rr: [npts, nsub] subtile maxes (f32, sorted-row order); exact
    (d2min, rank, n_flagged)."""
    npts = cfg["npts"]
    shard = cfg["shard"]
    sub = cfg["sub"]
    padl = cfg["padl"]
    ext_len = cfg["ext_len"]
    win = cfg["win"]

    n = npts
    loc = np.arange(n) % shard
    strip = loc // 128
    core = np.arange(n) // shard
    w0 = core * shard + strip * 128          # ext col of window start

    top2 = np.argpartition(-rr, 1, axis=1)[:, :2]
    cand = w0[:, None, None] + top2[:, :, None] * sub + np.arange(sub)[None, None, :]
    cand = cand.reshape(n, 2 * sub)
    tc = ext[cand]
    d2 = ((qs[:, None, :] - tc) ** 2).sum(-1)
    kk = np.argmin(d2, axis=1)
    dmin = d2[np.arange(n), kk]
    ecol = cand[np.arange(n), kk]

    # epsilon-aware bound over all non-candidate subtiles
    lb = a2[:, None] - (rr + eps)
    lb[np.arange(n)[:, None], top2] = np.inf
    flag_eps = lb.min(1) < dmin

    # x-gap optimality proof at window edges
    tx = ext[:, 0]
    wend = w0 + win
    safeL = np.where(w0 == 0, np.inf, qs[:, 0] - tx[np.maximum(w0 - 1, 0)])
    safeR = np.where(wend >= ext_len, np.inf,
                     tx[np.minimum(wend, ext_len - 1)] - qs[:, 0])
    safe = np.maximum(np.minimum(safeL, safeR), 0.0)
    flag = flag_eps | (dmin > safe * safe)

    fb = np.nonzero(flag)[0]
    if fb.size:
        dmin_fb, rank_fb = _exact_nn(qs[fb], t_sorted)
        dmin[fb] = dmin_fb
        ecol[fb] = rank_fb + padl
    rank = ecol - padl
    return dmin, rank, int(fb.size)


def _host_post(inputs, cfg, results, meta):
    npts = cfg["npts"]
    shard = cfg["shard"]
    n_strips = cfg["n_strips"]
    nsub = cfg["nsub"]
    win = cfg["win"]
    pairs = _pair_strips(cfg)
    v_pairs = set(cfg["v_pairs"])
    v_slot, s_slot = {}, {}
    for p in range(n_strips):
        if p in v_pairs:
            v_slot[p] = len(v_slot)
        else:
            s_slot[p] = len(s_slot)
    NVp = len(v_slot)
    NSp = len(s_slot)

    rr = {0: np.empty((npts, nsub), np.float32),
          1: np.empty((npts, nsub), np.float32)}
    eps = {0: np.empty((npts, nsub), np.float32),
           1: np.empty((npts, nsub), np.float32)}
    for c in range(N_CORES):
        outv = np.asarray(results[c]["out_vp"])          # [128, NVp*2*nsub+8]
        chamv = outv[:, :NVp * 2 * nsub].reshape(128, NVp, 2, nsub)
        chams = np.asarray(results[c]["cham_s"]).reshape(
            128, NSp, 2, win).astype(np.float32)
        for p in range(n_strips):
            for j, (side, k) in enumerate(pairs[p]):
                r0 = c * shard + k * 128
                nrow = min(128, shard - k * 128)
                if p in v_pairs:
                    blk = chamv[:nrow, v_slot[p], j, :]
                    e = 0.02 + 0.002 * np.abs(blk)
                else:
                    raw = chams[:nrow, s_slot[p], j, :].reshape(nrow, nsub, 32)
                    blk = raw.max(2)
                    e = 0.02 + 0.005 * np.abs(blk)
                rr[side][r0:r0 + nrow] = blk
                eps[side][r0:r0 + nrow] = e

    ps, gs = meta["ps"], meta["gs"]
    a2p = (ps * ps).sum(-1)
    a2g = (gs * gs).sum(-1)
    dA, rankA, nfA = _cham_side(cfg, rr[0], eps[0], ps, meta["g_ext"], gs, a2p)
    dB, _, nfB = _cham_side(cfg, rr[1], eps[1], gs, meta["p_ext"], ps, a2g)
    ch = dA.mean() + dB.mean()
    import os
    if os.environ.get("KERNEL_DEBUG"):
        print(f"[kernel] fallback rows: A={nfA} B={nfB}")

    pperm, gperm = meta["pperm"], meta["gperm"]
    idxA = np.empty(npts, np.int64)
    idxA[pperm] = gperm[np.clip(rankA, 0, npts - 1)]
    pn = inputs["pred_normals"][0].astype(np.float64)
    gn = inputs["gt_normals"][0].astype(np.float64)
    matched = gn[idxA]
    e_ = 1e-8
    num = (pn * matched).sum(-1)
    den = np.maximum(np.linalg.norm(pn, axis=-1), e_) * \
        np.maximum(np.linalg.norm(matched, axis=-1), e_)
    nrm = float(np.mean(1.0 - np.abs(num / den)))

    nvsub = NVp * 2 * nsub
    parts = np.stack([np.asarray(results[c]["out_vp"])[:, nvsub:nvsub + 8]
                      for c in range(N_CORES)])
    psum = parts.astype(np.float64).sum(axis=(0, 1))
    sdf = (psum[0] + psum[1]) / 200000.0
    num_add, cnt_add = _eik_host_corrections(cfg, meta)
    eik_num = psum[2] + num_add
    eik_cnt = psum[3] + cnt_add
    eik = (eik_num / eik_cnt) if eik_cnt > 0 else 0.0

    npairs = meta["npairs"]
    edge = ((psum[4] + meta["edge_extra"]) / npairs) if npairs > 0 else 0.0

    total = (SDF_W * sdf + EIK_W * eik + CH_W * ch + NORM_W * nrm +
             EDGE_W * edge + WT_W * meta["wt"])
    return np.asarray(np.float32(total))


def kernel(**inputs):
    from concourse.bass_utils import run_bass_kernel_spmd
    cfg = FULL_CFG
    nc = get_program()
    in_maps, meta = _host_prep(inputs, cfg)
    res = run_bass_kernel_spmd(nc, in_maps, core_ids=list(range(N_CORES)))
    return _host_post(inputs, cfg, res.results, meta)


# revision 27
# speedup vs baseline: 1.1073x; 1.1073x over previous
"""Trainium2 Bass kernel for nn_ClearMeshLoss (8-core SPMD), v4.

Strategy (v4):
  - chamfer + normal-consistency: both clouds sorted by x on host. Each core
    owns 1250 consecutive sorted query rows (10 strips of 128) per side.
    Each strip scores a rank-aligned window of W=512 sorted target columns
    (+-1e9 x sentinels) with the augmented matmul c = 2*a.b - |b|^2 in
    bf16 hi/lo (K=11).  Matmuls are 4-way row-tiled (tile_position row
    groups) so up to 4 strips stream concurrently through the PE.  Strips
    are paired (A_k with B_{k+1}) into 2-bank PSUM tiles; evacuation is
    split: V-pairs are max-reduced to 32-wide subtile maxes on DVE (f32),
    S-pairs are copied PSUM->SBUF as bf16 by ScalarE and DMA'd raw to HBM
    where the host computes the subtile maxes.  Host picks top-2 subtiles
    per row, recomputes those 64 candidate distances exactly, PROVES
    optimality via the x-gap bound + an epsilon-aware bound over all
    subtiles, and falls back to an exact KD-tree query for rows failing
    the proof.  Exact for this input.
  - edge loss: cross products / dots in bf16 on DVE with a component-
    duplicated [x,y,z,x,y] layout (rotations become plain slices);
    sqrt+relu on ScalarE (sqrt table preloaded early); host does the
    integer edge pairing.
  - sdf: clips/sub on DVE (bf16), abs/exp + accumulations on ScalarE.
  - eikonal: finite diffs on DVE from the same sdf tiles (no extra DMA);
    row-border pairs are patched exactly on host.
"""
import numpy as np
import ml_dtypes

BF16 = np.dtype(ml_dtypes.bfloat16)

# ---------------------------------------------------------------- constants
SDF_W, EIK_W, CH_W, NORM_W, EDGE_W, WT_W = 1.0, 0.1, 1.0, 0.5, 0.3, 0.2
TRUNC, SURF_W, DIH_THR = 0.1, 5.0, 0.5
SIGMA = TRUNC / 3.0

N_CORES = 8

FULL_CFG = dict(
    npts=10000,
    shard=1250,
    n_strips=10,          # strips of 128 rows per side
    win=512,
    sub=32,
    padl=192,             # left sentinels in ext array
    ext_len=192 + 10000 + 222,
    slice_w=9 * 128 + 512,    # 1664
    v_pairs=(2, 7),       # pair ids reduced on-device (DVE); rest raw bf16
    sdf_f=196,
    sdf_shard=25000,
    pair_f=120,
    pair_cap=8 * 128 * 120,
)
FULL_CFG["nsub"] = FULL_CFG["win"] // FULL_CFG["sub"]

_PROG_CACHE = {}


def _pair_strips(cfg):
    """pair p -> [(side,k) j=0, (side,k) j=1]; A_k paired with B_{k+1} so
    adjacent matmuls hit different PE row groups."""
    n = cfg["n_strips"]
    return [[(0, p), (1, (p + 1) % n)] for p in range(n)]


def build_program(cfg):
    from contextlib import ExitStack
    import concourse.bacc as bacc
    import concourse.bass as bass
    import concourse.tile as tile
    from concourse import mybir

    f32 = mybir.dt.float32
    bf16 = mybir.dt.bfloat16
    AX = mybir.AxisListType
    OP = mybir.AluOpType
    AF = mybir.ActivationFunctionType

    n_strips = cfg["n_strips"]
    win = cfg["win"]
    sub = cfg["sub"]
    nsub = cfg["nsub"]
    slice_w = cfg["slice_w"]
    sdf_f = cfg["sdf_f"]
    P = cfg["pair_f"]
    pairs = _pair_strips(cfg)
    v_pairs = set(cfg["v_pairs"])
    NVp = len(v_pairs)
    NSp = n_strips - NVp

    # group-g strip lists (strips s with s%4==g), per side
    g_strips = [[s for s in range(n_strips) if s % 4 == g] for g in range(4)]

    nc = bacc.Bacc("TRN2", target_bir_lowering=False)

    # ---- dram inputs ----
    d_lhs = [nc.dram_tensor(f"lhs_g{g}", [11, 2 * len(g_strips[g]) * 128],
                            bf16, kind="ExternalInput") for g in range(4)]
    d_rhs = nc.dram_tensor("rhs_ab", [11, 2 * slice_w], bf16, kind="ExternalInput")
    d_sdf = nc.dram_tensor("sdf_pg", [128, 2 * sdf_f], bf16, kind="ExternalInput")
    d_edge = nc.dram_tensor("edge_in", [128, 24 * P], bf16, kind="ExternalInput")

    # ---- dram outputs ----
    d_chams = nc.dram_tensor("cham_s", [128, NSp * 2 * win], bf16,
                             kind="ExternalOutput")
    # cham_v then part cols: 0 sdf_absdiff, 1 sdf_dead, 2 eik_num, 3 eik_cnt,
    # 4 edge_relu
    d_out = nc.dram_tensor("out_vp", [128, NVp * 2 * nsub + 8], f32,
                           kind="ExternalOutput")

    with tile.TileContext(nc) as tc, ExitStack() as ctx:
        sing = ctx.enter_context(tc.tile_pool(name="sing", bufs=1))
        epool = ctx.enter_context(tc.tile_pool(name="epool", bufs=1))
        spool = ctx.enter_context(tc.tile_pool(name="spool", bufs=1))
        psum = ctx.enter_context(tc.tile_pool(name="psum", bufs=4, space="PSUM"))

        NVs = NVp * 2 * nsub
        out_vp = sing.tile([128, NVs + 8], f32)

        def part_col(c):
            return out_vp[:, NVs + c:NVs + c + 1]

        chams_sz = [3, 3, NSp - 6]
        chams_t = [sing.tile([128, chams_sz[q], 2, win], bf16,
                             name=f"chams_t{q}") for q in range(3)]

        nc.vector.memset(out_vp[:, NVs:NVs + 8], 0.0)
        nbias = epool.tile([128, 1], f32)
        nc.vector.memset(nbias, -DIH_THR)

        # ---- input DMAs (all HWDGE queues; SWDGE pays a ~6us ucode load) ----
        lhs_sb = [sing.tile([128, 2 * len(g_strips[g]) * 128], bf16,
                            name=f"lhs_sb{g}") for g in range(4)]
        rhs_sb = [sing.tile([128, 2 * slice_w], bf16, name=f"rhs_sb{g}")
                  for g in range(4)]
        pl = epool.tile([128, 24, P], bf16)
        nc.scalar.dma_start(out=pl, in_=d_edge[:, :])
        for g in range(4):
            ng = len(g_strips[g])
            nc.sync.dma_start(out=rhs_sb[g][32 * g:32 * g + 11, :],
                              in_=d_rhs[:, :])
            nc.scalar.dma_start(
                out=lhs_sb[g][32 * g:32 * g + 11, 0:2 * ng * 128],
                in_=d_lhs[g][:, :])

        sdf_sb = spool.tile([128, 2 * sdf_f], bf16)
        sp = sdf_sb[:, 0:sdf_f]
        sg = sdf_sb[:, sdf_f:2 * sdf_f]
        nc.sync.dma_start(out=sdf_sb, in_=d_sdf[:, :])

        # ---- matmuls: 10 pairs x 2 strips, 4-way row-tiled ----
        ps_tiles = []
        for p in range(n_strips):
            ps = psum.tile([128, 2, win], f32)
            for j, (side, k) in enumerate(pairs[p]):
                g, t = k % 4, k // 4
                lcol = (side * len(g_strips[g]) + t) * 128
                rcol = side * slice_w + k * 128
                nc.tensor.matmul(ps[:, j, :],
                                 lhs_sb[g][32 * g:32 * g + 11, lcol:lcol + 128],
                                 rhs_sb[g][32 * g:32 * g + 11, rcol:rcol + win],
                                 start=True, stop=True,
                                 tile_position=(32 * g, 0))
            ps_tiles.append(ps)

        # ---- sdf elementwise (DVE bf16) ----
        prc = spool.tile([128, sdf_f], bf16)
        gc = spool.tile([128, sdf_f], bf16)
        nc.vector.tensor_scalar(out=prc, in0=sp, scalar1=TRUNC, scalar2=-TRUNC,
                                op0=OP.min, op1=OP.max)
        nc.vector.tensor_scalar(out=gc, in0=sg, scalar1=TRUNC, scalar2=-TRUNC,
                                op0=OP.min, op1=OP.max)
        diff = spool.tile([128, sdf_f], bf16)
        nc.vector.tensor_tensor(out=diff, in0=prc, in1=gc, op=OP.subtract)

        # Scalar: abs-diff accum, weights
        absdiff = spool.tile([128, sdf_f], bf16)
        nc.scalar.activation(out=absdiff, in_=diff, func=AF.Abs,
                             accum_out=part_col(0))
        absg = spool.tile([128, sdf_f], bf16)
        nc.scalar.activation(out=absg, in_=gc, func=AF.Abs)
        e4 = spool.tile([128, sdf_f], bf16)
        nc.scalar.activation(out=e4, in_=absg, func=AF.Exp, scale=-1.0 / SIGMA)

        # ---- eikonal (DVE; GpSimd only poisons the mask column) ----
        F1 = sdf_f - 1
        dx = spool.tile([128, F1], bf16)
        nc.vector.tensor_tensor(out=dx, in0=sp[:, 1:sdf_f], in1=sp[:, 0:F1],
                                op=OP.subtract)
        ndx = spool.tile([128, F1], bf16)
        nc.vector.tensor_scalar(out=ndx, in0=dx, scalar1=-1.0, scalar2=None,
                                op0=OP.mult)
        absdx = spool.tile([128, F1], bf16)
        nc.vector.tensor_tensor(out=absdx, in0=dx, in1=ndx, op=OP.max)
        # poison column j=107 so the shard-boundary pair (row 127) is masked
        # out; host exactly re-adds the 127 real pairs this also kills
        nc.gpsimd.memset(absg[:, 107:108], 1.0)

        # ======== evacuation + edge, interleaved for engine overlap ========
        v_slot, s_slot = {}, {}
        for p in range(n_strips):
            if p in v_pairs:
                v_slot[p] = len(v_slot)
            else:
                s_slot[p] = len(s_slot)

        def evac(p):
            ps = ps_tiles[p]
            if p in v_pairs:
                vi = v_slot[p]
                ps_ap = ps[:, :, :]
                ps4d = bass.AP(tensor=ps_ap.tensor, offset=ps_ap.offset,
                               ap=[ps_ap.ap[0], [win, 2], [sub, nsub],
                                   [1, sub]])
                nc.vector.tensor_reduce(
                    out=out_vp[:, vi * 2 * nsub:(vi + 1) * 2 * nsub],
                    in_=ps4d, axis=AX.X, op=OP.max)
            else:
                si = s_slot[p]
                q, li = (0, si) if si < 3 else ((1, si - 3) if si < 6
                                                else (2, si - 6))
                nc.scalar.activation(out=chams_t[q][:, li, :, :],
                                     in_=ps[:, :, :], func=AF.Copy)

        # edge tiles; E5 holds each edge vector with components [x,y,z,x,y]
        # so rot1/rot2 are plain slices (comps 1:4 / 2:5).
        E5 = epool.tile([128, 4, 5, P], bf16)     # e1A,e2A,e1B,e2B
        T1 = epool.tile([128, 2, 3, P], bf16)
        T2 = epool.tile([128, 2, 3, P], bf16)
        NN = epool.tile([128, 2, 3, P], bf16)
        SS = epool.tile([128, 3, 3, P], bf16)     # [na^2, nb^2, na*nb] comps
        A1 = epool.tile([128, 3, P], bf16)
        DOTS = epool.tile([128, 3, P], bf16)
        den2 = epool.tile([128, P], f32)
        rs = epool.tile([128, P], f32)
        cosb = epool.tile([128, P], f32)
        relu_d = epool.tile([128, P], f32)

        plb = pl[:, :, :]
        e5b = E5[:, :, :, :]

        def pl_ap(plane0, ncomp):
            return bass.AP(tensor=plb.tensor, offset=plb.offset + plane0 * P,
                           ap=[plb.ap[0], [3 * P, 4], [P, ncomp], [1, P]])

        def e5_ap(comp0, ncomp, vstep=1, v0=0, nvec=4):
            return bass.AP(tensor=e5b.tensor,
                           offset=e5b.offset + (v0 * 5 + comp0) * P,
                           ap=[e5b.ap[0], [vstep * 5 * P, nvec], [P, ncomp],
                               [1, P]])

        edge_ops = []
        # plane order: v1A v2A v1B v2B (0:12) then v0A v0A v0B v0B (12:24)
        edge_ops.append(lambda: nc.vector.tensor_tensor(
            out=e5_ap(0, 3), in0=pl_ap(0, 3), in1=pl_ap(12, 3), op=OP.subtract))
        edge_ops.append(lambda: nc.vector.tensor_tensor(
            out=e5_ap(3, 2), in0=pl_ap(0, 2), in1=pl_ap(12, 2), op=OP.subtract))
        # crosses: na = e1A_r1*e2A_r2 - e1A_r2*e2A_r1 ; nb likewise
        edge_ops.append(lambda: nc.vector.tensor_tensor(
            out=T1[:, :, :, :], in0=e5_ap(1, 3, 2, 0, 2),
            in1=e5_ap(2, 3, 2, 1, 2), op=OP.mult))
        edge_ops.append(lambda: nc.vector.tensor_tensor(
            out=T2[:, :, :, :], in0=e5_ap(2, 3, 2, 0, 2),
            in1=e5_ap(1, 3, 2, 1, 2), op=OP.mult))
        edge_ops.append(lambda: nc.vector.tensor_tensor(
            out=NN[:, :, :, :], in0=T1[:, :, :, :], in1=T2[:, :, :, :],
            op=OP.subtract))
        # dots
        edge_ops.append(lambda: nc.vector.tensor_tensor(
            out=SS[:, 0:2, :, :], in0=NN[:, :, :, :], in1=NN[:, :, :, :],
            op=OP.mult))
        edge_ops.append(lambda: nc.vector.tensor_tensor(
            out=SS[:, 2, :, :], in0=NN[:, 0, :, :], in1=NN[:, 1, :, :],
            op=OP.mult))
        edge_ops.append(lambda: nc.vector.tensor_tensor(
            out=A1[:, :, :], in0=SS[:, :, 0, :], in1=SS[:, :, 1, :], op=OP.add))
        edge_ops.append(lambda: nc.vector.tensor_tensor(
            out=DOTS[:, :, :], in0=A1[:, :, :], in1=SS[:, :, 2, :], op=OP.add))
        edge_ops.append(lambda: nc.vector.tensor_tensor(
            out=den2, in0=DOTS[:, 0, :], in1=DOTS[:, 1, :], op=OP.mult))

        # eik/sdf accumulator chain as interleavable ops (DVE)
        t_ = spool.tile([128, F1], bf16)
        mask = spool.tile([128, F1], bf16)
        tm = spool.tile([128, F1], bf16)
        deadd = spool.tile([128, sdf_f], bf16)
        eikd = spool.tile([128, F1], bf16)
        acc_ops = [
            lambda: nc.vector.tensor_scalar(out=t_, in0=absdx, scalar1=-1.0,
                                            scalar2=None, op0=OP.add),
            lambda: nc.vector.tensor_scalar(out=mask, in0=absg[:, 0:F1],
                                            scalar1=TRUNC, scalar2=None,
                                            op0=OP.is_lt),
            lambda: nc.vector.tensor_tensor(out=tm, in0=t_, in1=mask,
                                            op=OP.mult),
            lambda: nc.vector.tensor_reduce(out=part_col(3), in_=mask,
                                            axis=AX.X, op=OP.add),
            lambda: nc.vector.scalar_tensor_tensor(
                out=deadd, in0=e4, scalar=SURF_W - 1.0, in1=absdiff,
                op0=OP.mult, op1=OP.mult, accum_out=part_col(1)),
            lambda: nc.vector.scalar_tensor_tensor(
                out=eikd, in0=tm, scalar=1.0, in1=t_, op0=OP.mult,
                op1=OP.mult, accum_out=part_col(2)),
        ]

        # interleave: pair evacuations with edge + accumulator ops slotted in
        fill_iter = iter(edge_ops + acc_ops)
        for p in range(n_strips):
            evac(p)
            if p >= 2:
                for _ in range(2):
                    op = next(fill_iter, None)
                    if op is not None:
                        op()
            # stage cham_s out as chunks complete
            if p == 3:
                nc.sync.dma_start(out=d_chams[:, 0:3 * 2 * win],
                                  in_=chams_t[0][:, :, :, :])
            elif p == 6:
                nc.sync.dma_start(out=d_chams[:, 3 * 2 * win:6 * 2 * win],
                                  in_=chams_t[1][:, :, :, :])
        for op in fill_iter:
            op()
        nc.sync.dma_start(out=d_chams[:, 6 * 2 * win:NSp * 2 * win],
                          in_=chams_t[2][:, :, :, :])

        # edge tail: rs = rsqrt(den2) via int-bit trick + one Newton step (DVE)
        i32 = mybir.dt.int32
        q1 = epool.tile([128, P], i32)
        q2 = epool.tile([128, P], i32)
        y0 = epool.tile([128, P], f32)
        nc.vector.tensor_scalar(out=q1, in0=den2[:, :].bitcast(i32), scalar1=1,
                                scalar2=None, op0=OP.logical_shift_right)
        nc.vector.tensor_scalar(out=q2, in0=q1, scalar1=-1, scalar2=None,
                                op0=OP.bitwise_xor)
        nc.vector.tensor_scalar(out=y0[:, :].bitcast(i32), in0=q2,
                                scalar1=0x5f3759df + 1, scalar2=None,
                                op0=OP.add)
        hh = epool.tile([128, P], f32)
        nc.vector.tensor_tensor(out=hh, in0=y0, in1=y0, op=OP.mult)
        h2 = epool.tile([128, P], f32)
        nc.vector.tensor_tensor(out=h2, in0=hh, in1=den2, op=OP.mult)
        uu = epool.tile([128, P], f32)
        nc.vector.tensor_scalar(out=uu, in0=h2, scalar1=-0.5, scalar2=1.5,
                                op0=OP.mult, op1=OP.add)
        nc.vector.tensor_tensor(out=rs, in0=uu, in1=y0, op=OP.mult)
        nc.vector.tensor_tensor(out=cosb, in0=DOTS[:, 2, :], in1=rs, op=OP.mult)
        nc.scalar.activation(out=relu_d, in_=cosb, func=AF.Relu,
                             bias=nbias[:, 0:1], accum_out=part_col(4))

        nc.sync.dma_start(out=d_out[:, :], in_=out_vp[:, :])

    nc.compile()
    return nc


def get_program(cfg_key="full"):
    if cfg_key not in _PROG_CACHE:
        _PROG_CACHE[cfg_key] = build_program(FULL_CFG)
    return _PROG_CACHE[cfg_key]


# ================================================================== host side
def _hi_lo(x):
    h = x.astype(BF16)
    l = (x - h.astype(np.float64)).astype(BF16)
    return h, l


def _build_lhs(a):
    """a: [n,3] fp64 -> [11,n] bf16 rows [ah3, ah3, al3, 1, 1]."""
    ah, al = _hi_lo(a)
    ones = np.ones((1, a.shape[0]), BF16)
    return np.ascontiguousarray(np.concatenate([ah.T, ah.T, al.T, ones, ones], 0))


def _build_rhs(b):
    """b: [m,3] fp64 -> [11,m] bf16 rows [2bh3, 2bl3, 2bh3, -sh, -sl]."""
    bh = b.astype(BF16)
    bl2 = (2.0 * (b - bh.astype(np.float64))).astype(BF16)
    bh2 = (2.0 * bh.astype(np.float64)).astype(BF16)
    s = (b * b).sum(-1)
    sh = s.astype(BF16)
    sl = (s - sh.astype(np.float64)).astype(BF16)
    neg_sh = (-sh.astype(np.float64)).astype(BF16)
    neg_sl = (-sl.astype(np.float64)).astype(BF16)
    return np.ascontiguousarray(
        np.concatenate([bh2.T, bl2.T, bh2.T, neg_sh[None], neg_sl[None]], 0))


def _pad_rows(x, n):
    out = np.zeros((n, 3))
    out[:x.shape[0]] = x
    return out


def _host_prep(inputs, cfg):
    np_f32 = np.float32
    npts = cfg["npts"]
    shard = cfg["shard"]
    n_strips = cfg["n_strips"]
    slice_w = cfg["slice_w"]
    padl = cfg["padl"]
    ext_len = cfg["ext_len"]
    sdf_f = cfg["sdf_f"]
    sdf_shard = cfg["sdf_shard"]
    P = cfg["pair_f"]

    pred_pts = np.asarray(inputs["pred_points"][0], dtype=np.float64)
    gt_pts = np.asarray(inputs["gt_points"][0], dtype=np.float64)

    pperm = np.argsort(pred_pts[:, 0], kind="stable")
    gperm = np.argsort(gt_pts[:, 0], kind="stable")
    ps = pred_pts[pperm]
    gs = gt_pts[gperm]

    def make_ext(sorted_pts):
        ext = np.empty((ext_len, 3))
        ext[:padl] = [-1e9, 0.0, 0.0]
        ext[padl:padl + npts] = sorted_pts
        ext[padl + npts:] = [1e9, 0.0, 0.0]
        return ext

    g_ext = make_ext(gs)
    p_ext = make_ext(ps)
    rhs_gt = _build_rhs(g_ext)     # [11, ext_len]
    rhs_pr = _build_rhs(p_ext)

    pred_sdf = inputs["pred_sdf"].reshape(-1).astype(np_f32)
    gt_sdf = inputs["gt_sdf"].reshape(-1).astype(np_f32)

    # --- edge pairing on host (int32 faces only) ---
    verts = np.asarray(inputs["extracted_vertices"], dtype=np_f32)
    faces = np.asarray(inputs["extracted_faces"], dtype=np.int64)
    V = verts.shape[0]
    Fn = faces.shape[0]
    a = faces
    b = np.roll(faces, -1, axis=1)
    lo = np.minimum(a, b)
    hi = np.maximum(a, b)
    key = (lo * V + hi).reshape(-1)
    fid = np.repeat(np.arange(Fn, dtype=np.int64), 3)
    order = np.argsort(key, kind="stable")
    k = key[order]
    f = fid[order]
    same_next = k[:-1] == k[1:]
    prev = np.concatenate([[False], same_next[:-1]])
    nxt = np.concatenate([same_next[1:], [False]])
    is_pair = same_next & ~prev & ~nxt
    pos = np.nonzero(is_pair)[0]
    fa = f[pos]
    fb = f[pos + 1]
    npairs = int(pos.shape[0])
    is_start = np.concatenate([[True], k[1:] != k[:-1]])
    starts = np.nonzero(is_start)[0]
    run_len = np.diff(np.concatenate([starts, [k.shape[0]]]))
    total_unique = int(starts.shape[0])
    bad = int((run_len != 2).sum())
    wt = (bad / total_unique) if total_unique > 0 else 0.0

    pair_cap = cfg["pair_cap"]
    n_dev = min(npairs, pair_cap)
    # plane order: v1A v2A v1B v2B | v0A v0A v0B v0B (each 3 comps)
    planes = np.zeros((24, pair_cap), np_f32)
    if n_dev > 0:
        va = verts[faces[fa[:n_dev]]]     # [n,3(vert),3(comp)]
        vb = verts[faces[fb[:n_dev]]]
        planes[0:3, :n_dev] = va[:, 1].T
        planes[3:6, :n_dev] = va[:, 2].T
        planes[6:9, :n_dev] = vb[:, 1].T
        planes[9:12, :n_dev] = vb[:, 2].T
        planes[12:15, :n_dev] = va[:, 0].T
        planes[15:18, :n_dev] = va[:, 0].T
        planes[18:21, :n_dev] = vb[:, 0].T
        planes[21:24, :n_dev] = vb[:, 0].T
    edge_extra = 0.0
    if npairs > pair_cap:
        va = verts[faces[fa[pair_cap:]]]
        vb = verts[faces[fb[pair_cap:]]]
        na = np.cross(va[:, 1] - va[:, 0], va[:, 2] - va[:, 0])
        nb = np.cross(vb[:, 1] - vb[:, 0], vb[:, 2] - vb[:, 0])
        na /= np.maximum(np.linalg.norm(na, axis=-1, keepdims=True), 1e-12)
        nb /= np.maximum(np.linalg.norm(nb, axis=-1, keepdims=True), 1e-12)
        cosv = (na * nb).sum(-1)
        edge_extra = float(np.maximum(cosv - DIH_THR, 0.0).sum())
    planes_bf = planes.astype(BF16)
    planes8 = planes_bf.reshape(24, N_CORES, 128, P).transpose(1, 2, 0, 3)

    g_strips = [[s for s in range(n_strips) if s % 4 == g] for g in range(4)]

    in_maps = []
    sdf_tiles_p, sdf_tiles_g = [], []
    for c in range(N_CORES):
        lhs_a = _build_lhs(_pad_rows(ps[c * shard:(c + 1) * shard], 128 * n_strips))
        lhs_b = _build_lhs(_pad_rows(gs[c * shard:(c + 1) * shard], 128 * n_strips))
        im = {}
        for g in range(4):
            ng = len(g_strips[g])
            blk = np.empty((11, 2 * ng * 128), BF16)
            for side, lhs in ((0, lhs_a), (1, lhs_b)):
                for t, s in enumerate(g_strips[g]):
                    blk[:, (side * ng + t) * 128:(side * ng + t + 1) * 128] = \
                        lhs[:, s * 128:(s + 1) * 128]
            im[f"lhs_g{g}"] = np.ascontiguousarray(blk)
        im["rhs_ab"] = np.ascontiguousarray(np.concatenate(
            [rhs_gt[:, c * shard:c * shard + slice_w],
             rhs_pr[:, c * shard:c * shard + slice_w]], axis=1))

        spd = np.full(128 * sdf_f, 1e9, np_f32)
        sgd = np.full(128 * sdf_f, 1e9, np_f32)
        sl = pred_sdf[c * sdf_shard:(c + 1) * sdf_shard]
        spd[:sl.shape[0]] = sl
        sgd[:sl.shape[0]] = gt_sdf[c * sdf_shard:(c + 1) * sdf_shard]
        spd_bf = spd.astype(BF16).reshape(128, sdf_f)
        sgd_bf = sgd.astype(BF16).reshape(128, sdf_f)
        im["sdf_pg"] = np.ascontiguousarray(
            np.concatenate([spd_bf, sgd_bf], axis=1))
        sdf_tiles_p.append(spd_bf)
        sdf_tiles_g.append(sgd_bf)

        im["edge_in"] = np.ascontiguousarray(planes8[c].reshape(128, 24 * P))
        in_maps.append(im)

    meta = dict(npairs=npairs, wt=wt, edge_extra=edge_extra,
                pperm=pperm, gperm=gperm, ps=ps, gs=gs,
                p_ext=p_ext, g_ext=g_ext,
                sdf_p=sdf_tiles_p, sdf_g=sdf_tiles_g,
                pred_sdf=pred_sdf, gt_sdf=gt_sdf)
    return in_maps, meta


def _eik_host_corrections(cfg, meta):
    """Row-border dx pairs the device skips + the poisoned mask column,
    computed with the same bf16-input/f32-arith convention."""
    sdf_f, sdf_shard = cfg["sdf_f"], cfg["sdf_shard"]
    n_batch = 100000
    n_tot = 200000
    num_add = 0.0
    cnt_add = 0.0
    for c in range(N_CORES):
        spd = meta["sdf_p"][c].reshape(-1).astype(np.float32)
        sgd = meta["sdf_g"][c].reshape(-1).astype(np.float32)
        # (a) row borders (L = 196p+195) + poisoned column (L = 196p+107),
        #     p in [0, 126]
        p = np.arange(127)
        L = np.concatenate([sdf_f * p + (sdf_f - 1), sdf_f * p + 107])
        ok = L + 1 <= sdf_shard - 1
        L = L[ok]
        i_glob = c * sdf_shard + L
        valid = (i_glob % n_batch) != n_batch - 1
        dxv = spd[L + 1] - spd[L]
        tv = np.abs(dxv) - 1.0
        mk = (np.abs(sgd[L]) < TRUNC) & valid
        num_add += float((tv * tv * mk).sum())
        cnt_add += float(mk.sum())
        # (b) poisoned slot L=24999: pair crosses into next core's shard
        L = sdf_shard - 1
        i_glob = c * sdf_shard + L
        if i_glob + 1 < n_tot and (i_glob % n_batch) != n_batch - 1:
            nxt2 = meta["pred_sdf"][(c + 1) * sdf_shard]
            nxt2 = np.float32(np.asarray(nxt2, np.float32).astype(BF16))
            dxv = nxt2 - spd[L]
            tv = np.abs(dxv) - 1.0
            mk = np.abs(sgd[L]) < TRUNC
            if mk:
                num_add += float(tv * tv)
                cnt_add += 1.0
    return num_add, cnt_add


def _exact_nn(q, t_sorted):
    try:
        from scipy.spatial import cKDTree
        tree = cKDTree(t_sorted)
        d, idx = tree.query(q, k=1)
        return d * d, idx
    except Exception:
        n = q.shape[0]
        dm = np.empty(n)
        im = np.empty(n, np.int64)
        B = 512
        for i in range(0, n, B):
            d2 = ((q[i:i + B, None, :] - t_sorted[None, :, :]) ** 2).sum(-1)
            im[i:i + B] = np.argmin(d2, 1)
            dm[i:i + B] = d2[np.arange(d2.shape[0]), im[i:i + B]]
        return dm, im


def _cham_side(cfg, rr, eps, qs, ext, t_sorted, a2):
    """rr: [npts, nsub] subtile maxes (f32, sorted-row order); exact
    (d2min, rank, n_flagged)."""
    npts = cfg["npts"]
    shard = cfg["shard"]
    sub = cfg["sub"]
    padl = cfg["padl"]
    ext_len = cfg["ext_len"]
    win = cfg["win"]

    n = npts
    loc = np.arange(n) % shard
    strip = loc // 128
    core = np.arange(n) // shard
    w0 = core * shard + strip * 128          # ext col of window start

    top2 = np.argpartition(-rr, 1, axis=1)[:, :2]
    cand = w0[:, None, None] + top2[:, :, None] * sub + np.arange(sub)[None, None, :]
    cand = cand.reshape(n, 2 * sub)
    tc = ext[cand]
    d2 = ((qs[:, None, :] - tc) ** 2).sum(-1)
    kk = np.argmin(d2, axis=1)
    dmin = d2[np.arange(n), kk]
    ecol = cand[np.arange(n), kk]

    # epsilon-aware bound over all non-candidate subtiles
    lb = a2[:, None] - (rr + eps)
    lb[np.arange(n)[:, None], top2] = np.inf
    flag_eps = lb.min(1) < dmin

    # x-gap optimality proof at window edges
    tx = ext[:, 0]
    wend = w0 + win
    safeL = np.where(w0 == 0, np.inf, qs[:, 0] - tx[np.maximum(w0 - 1, 0)])
    safeR = np.where(wend >= ext_len, np.inf,
                     tx[np.minimum(wend, ext_len - 1)] - qs[:, 0])
    safe = np.maximum(np.minimum(safeL, safeR), 0.0)
    flag = flag_eps | (dmin > safe * safe)

    fb = np.nonzero(flag)[0]
    if fb.size:
        dmin_fb, rank_fb = _exact_nn(qs[fb], t_sorted)
        dmin[fb] = dmin_fb
        ecol[fb] = rank_fb + padl
    rank = ecol - padl
    return dmin, rank, int(fb.size)


def _host_post(inputs, cfg, results, meta):
    npts = cfg["npts"]
    shard = cfg["shard"]
    n_strips = cfg["n_strips"]
    nsub = cfg["nsub"]
    win = cfg["win"]
    pairs = _pair_strips(cfg)
    v_pairs = set(cfg["v_pairs"])
    v_slot, s_slot = {}, {}
    for p in range(n_strips):
        if p in v_pairs:
            v_slot[p] = len(v_slot)
        else:
            s_slot[p] = len(s_slot)
    NVp = len(v_slot)
    NSp = len(s_slot)

    rr = {0: np.empty((npts, nsub), np.float32),
          1: np.empty((npts, nsub), np.float32)}
    eps = {0: np.empty((npts, nsub), np.float32),
           1: np.empty((npts, nsub), np.float32)}
    for c in range(N_CORES):
        outv = np.asarray(results[c]["out_vp"])          # [128, NVp*2*nsub+8]
        chamv = outv[:, :NVp * 2 * nsub].reshape(128, NVp, 2, nsub)
        chams = np.asarray(results[c]["cham_s"]).reshape(
            128, NSp, 2, win).astype(np.float32)
        for p in range(n_strips):
            for j, (side, k) in enumerate(pairs[p]):
                r0 = c * shard + k * 128
                nrow = min(128, shard - k * 128)
                if p in v_pairs:
                    blk = chamv[:nrow, v_slot[p], j, :]
                    e = 0.02 + 0.002 * np.abs(blk)
                else:
                    raw = chams[:nrow, s_slot[p], j, :].reshape(nrow, nsub, 32)
                    blk = raw.max(2)
                    e = 0.02 + 0.005 * np.abs(blk)
                rr[side][r0:r0 + nrow] = blk
                eps[side][r0:r0 + nrow] = e

    ps, gs = meta["ps"], meta["gs"]
    a2p = (ps * ps).sum(-1)
    a2g = (gs * gs).sum(-1)
    dA, rankA, nfA = _cham_side(cfg, rr[0], eps[0], ps, meta["g_ext"], gs, a2p)
    dB, _, nfB = _cham_side(cfg, rr[1], eps[1], gs, meta["p_ext"], ps, a2g)
    ch = dA.mean() + dB.mean()
    import os
    if os.environ.get("KERNEL_DEBUG"):
        print(f"[kernel] fallback rows: A={nfA} B={nfB}")

    pperm, gperm = meta["pperm"], meta["gperm"]
    idxA = np.empty(npts, np.int64)
    idxA[pperm] = gperm[np.clip(rankA, 0, npts - 1)]
    pn = inputs["pred_normals"][0].astype(np.float64)
    gn = inputs["gt_normals"][0].astype(np.float64)
    matched = gn[idxA]
    e_ = 1e-8
    num = (pn * matched).sum(-1)
    den = np.maximum(np.linalg.norm(pn, axis=-1), e_) * \
        np.maximum(np.linalg.norm(matched, axis=-1), e_)
    nrm = float(np.mean(1.0 - np.abs(num / den)))

    nvsub = NVp * 2 * nsub
    parts = np.stack([np.asarray(results[c]["out_vp"])[:, nvsub:nvsub + 8]
                      for c in range(N_CORES)])
    psum = parts.astype(np.float64).sum(axis=(0, 1))
    sdf = (psum[0] + psum[1]) / 200000.0
    num_add, cnt_add = _eik_host_corrections(cfg, meta)
    eik_num = psum[2] + num_add
    eik_cnt = psum[3] + cnt_add
    eik = (eik_num / eik_cnt) if eik_cnt > 0 else 0.0

    npairs = meta["npairs"]
    edge = ((psum[4] + meta["edge_extra"]) / npairs) if npairs > 0 else 0.0

    total = (SDF_W * sdf + EIK_W * eik + CH_W * ch + NORM_W * nrm +
             EDGE_W * edge + WT_W * meta["wt"])
    return np.asarray(np.float32(total))


def kernel(**inputs):
    from concourse.bass_utils import run_bass_kernel_spmd
    cfg = FULL_CFG
    nc = get_program()
    in_maps, meta = _host_prep(inputs, cfg)
    res = run_bass_kernel_spmd(nc, in_maps, core_ids=list(range(N_CORES)))
    return _host_post(inputs, cfg, res.results, meta)
